# revision 10
# baseline (speedup 1.0000x reference)
"""Cross-attention layer kernel for Trainium2 (8 NeuronCores, data-parallel over batch).

Per-core computation (batch element b):
  Q_ = conv3(Q, wq@wd) ; K_ = conv3(K, wk@wd) ; V_ = conv3(V, wv@wd)   (conv1+conv3 fused)
  S^T = K_ @ Q_^T  (m on partitions, l on free dim)
  expS = exp(S^T)  (no max subtraction; |S| <~ 45 is safe in fp32)
  [U; rowsum] = [V_ | ones]^T @ expS   (softmax denominator fused into the AV matmul)
  out^T = U / rowsum ;  y^T = wo^T @ out^T + bo
Host side: transposes + weight folding; device gets channel-major tensors.
"""

import numpy as np

import concourse.bass as bass
import concourse.tile as tile
from concourse import bacc, mybir
from concourse.bass_utils import run_bass_kernel_spmd

B, L, C = 8, 4096, 64
NCORES = 8
G = 512            # l-group width (columns of S^T per pass)
NG = L // G        # 8 l-groups
MC = 128           # m-chunk height
NMC = L // MC      # 32 m-chunks
F32 = mybir.dt.float32
EXP = mybir.ActivationFunctionType.Exp

# staging group sizes (chunks per exp-activation); 3 banks + 3 banks + 2 U banks = 8 PSUM banks
GROUPS = [3] * 10 + [2]
assert sum(GROUPS) == NMC


def build_program(dbg=False):
    nc = bacc.Bacc("TRN2", target_bir_lowering=False, debug=False, num_devices=NCORES)
    if dbg:
        dq_d = nc.dram_tensor("dq", [128, L], F32, kind="ExternalOutput")
        dk_d = nc.dram_tensor("dk", [128, L], F32, kind="ExternalOutput")
        dv_d = nc.dram_tensor("dv", [128, NMC * (C + 1)], F32, kind="ExternalOutput")
        des_d = nc.dram_tensor("des", [128, 3 * G], F32, kind="ExternalOutput")
        dus_d = nc.dram_tensor("dus", [65, G], F32, kind="ExternalOutput")
        dr_d = nc.dram_tensor("dr", [64, G], F32, kind="ExternalOutput")

    qt_d = nc.dram_tensor("qt", [C, L + 2], F32, kind="ExternalInput")
    kt_d = nc.dram_tensor("kt", [C, L + 2], F32, kind="ExternalInput")
    vt_d = nc.dram_tensor("vt", [C, L + 2], F32, kind="ExternalInput")
    wq_d = nc.dram_tensor("wq3", [128, 3, 128], F32, kind="ExternalInput")
    wk_d = nc.dram_tensor("wk3", [128, 3, 128], F32, kind="ExternalInput")
    wv_d = nc.dram_tensor("wv3", [128, 3, C], F32, kind="ExternalInput")
    wo_d = nc.dram_tensor("wo2", [C, C], F32, kind="ExternalInput")
    bq_d = nc.dram_tensor("bq2", [128, 1], F32, kind="ExternalInput")
    bk_d = nc.dram_tensor("bk2", [128, 1], F32, kind="ExternalInput")
    bv_d = nc.dram_tensor("bvb", [128, C], F32, kind="ExternalInput")
    bo_d = nc.dram_tensor("bot", [C, 1], F32, kind="ExternalInput")
    eq_d = nc.dram_tensor("eq2", [128, 2], F32, kind="ExternalInput")
    ek_d = nc.dram_tensor("ek2", [128, 2], F32, kind="ExternalInput")
    ev_d = nc.dram_tensor("ev2", [2, C], F32, kind="ExternalInput")
    yt_d = nc.dram_tensor("yt", [C, L], F32, kind="ExternalOutput")

    with tile.TileContext(nc) as tc:
        with tc.tile_pool(name="persist", bufs=1) as per:
            qin = per.tile([128, L + 2], F32)
            kin = per.tile([128, L + 2], F32)
            vin = per.tile([128, L + 2], F32)
            # duplicate channel-major inputs into both partition halves (row tiling)
            for dst, src in ((qin, qt_d), (kin, kt_d), (vin, vt_d)):
                nc.sync.dma_start(out=dst[0:C, :], in_=src[:, :])
                nc.sync.dma_start(out=dst[C : 2 * C, :], in_=src[:, :])
            wq_sb = per.tile([128, 3, 128], F32)
            nc.sync.dma_start(out=wq_sb, in_=wq_d[:, :, :])
            wk_sb = per.tile([128, 3, 128], F32)
            nc.sync.dma_start(out=wk_sb, in_=wk_d[:, :, :])
            wv_sb = per.tile([128, 3, C], F32)
            nc.sync.dma_start(out=wv_sb, in_=wv_d[:, :, :])
            wo_sb = per.tile([C, C], F32)
            nc.sync.dma_start(out=wo_sb, in_=wo_d[:, :])
            bq_sb = per.tile([128, 1], F32)
            nc.sync.dma_start(out=bq_sb, in_=bq_d[:, :])
            bk_sb = per.tile([128, 1], F32)
            nc.sync.dma_start(out=bk_sb, in_=bk_d[:, :])
            bv_sb = per.tile([128, C], F32)
            nc.sync.dma_start(out=bv_sb, in_=bv_d[:, :])
            bo_sb = per.tile([C, 1], F32)
            nc.sync.dma_start(out=bo_sb, in_=bo_d[:, :])
            eq_sb = per.tile([128, 2], F32)
            nc.sync.dma_start(out=eq_sb, in_=eq_d[:, :])
            ek_sb = per.tile([128, 2], F32)
            nc.sync.dma_start(out=ek_sb, in_=ek_d[:, :])
            ev_sb = per.tile([128, C], F32)
            nc.vector.memset(ev_sb[:, :], 0.0)
            nc.sync.dma_start(out=ev_sb[0:1, :], in_=ev_d[0:1, :])
            nc.sync.dma_start(out=ev_sb[127:128, :], in_=ev_d[1:2, :])

            qT = per.tile([128, L], F32)   # Q_^T, duplicated halves
            kT = per.tile([128, L], F32)   # K_^T, duplicated halves
            vrow = per.tile([128, NMC, C + 1], F32)  # V_ row-major chunks + ones col

            # ---------------- projections ----------------
            with tc.tile_pool(name="pqk", bufs=4, space="PSUM") as pqk, tc.tile_pool(
                name="pv", bufs=4, space="PSUM"
            ) as pv:
                for xin, w_sb, b_sb, xT in (
                    (qin, wq_sb, bq_sb, qT),
                    (kin, wk_sb, bk_sb, kT),
                ):
                    for g0 in range(0, NG, 2):
                        psA = pqk.tile([128, G], F32, tag="qk", name="psA")
                        psB = pqk.tile([128, G], F32, tag="qk", name="psB")
                        for k in range(3):
                            nc.tensor.matmul(
                                psA,
                                lhsT=w_sb[0:64, k, :],
                                rhs=xin[0:64, g0 * G + k : g0 * G + k + G],
                                start=(k == 0),
                                stop=(k == 2),
                                tile_position=(0, 0),
                            )
                            nc.tensor.matmul(
                                psB,
                                lhsT=w_sb[64:128, k, :],
                                rhs=xin[64:128, (g0 + 1) * G + k : (g0 + 1) * G + k + G],
                                start=(k == 0),
                                stop=(k == 2),
                                tile_position=(64, 0),
                            )
                        nc.vector.tensor_scalar_add(
                            out=xT[:, g0 * G : (g0 + 1) * G], in0=psA, scalar1=b_sb
                        )
                        nc.vector.tensor_scalar_add(
                            out=xT[:, (g0 + 1) * G : (g0 + 2) * G], in0=psB, scalar1=b_sb
                        )
                # conv edge corrections (pad column saw folded conv1 bias)
                nc.vector.tensor_scalar_add(
                    out=qT[:, 0:1], in0=qT[:, 0:1], scalar1=eq_sb[:, 0:1]
                )
                nc.vector.tensor_scalar_add(
                    out=qT[:, L - 1 : L], in0=qT[:, L - 1 : L], scalar1=eq_sb[:, 1:2]
                )
                nc.vector.tensor_scalar_add(
                    out=kT[:, 0:1], in0=kT[:, 0:1], scalar1=ek_sb[:, 0:1]
                )
                nc.vector.tensor_scalar_add(
                    out=kT[:, L - 1 : L], in0=kT[:, L - 1 : L], scalar1=ek_sb[:, 1:2]
                )

                # V_ row-major conv (shifted-window lhsT), paired row tiles
                nc.vector.memset(vrow[:, :, C : C + 1], 1.0)
                for c0 in range(0, NMC, 2):
                    pvA = pv.tile([128, C], F32, tag="v", name="pvA")
                    pvB = pv.tile([128, C], F32, tag="v", name="pvB")
                    for k in range(3):
                        nc.tensor.matmul(
                            pvA,
                            lhsT=vin[0:64, c0 * MC + k : c0 * MC + k + MC],
                            rhs=wv_sb[0:64, k, :],
                            start=(k == 0),
                            stop=(k == 2),
                            tile_position=(0, 0),
                        )
                        nc.tensor.matmul(
                            pvB,
                            lhsT=vin[64:128, (c0 + 1) * MC + k : (c0 + 1) * MC + k + MC],
                            rhs=wv_sb[64:128, k, :],
                            start=(k == 0),
                            stop=(k == 2),
                            tile_position=(64, 0),
                        )
                    nc.vector.tensor_add(out=vrow[:, c0, 0:C], in0=pvA, in1=bv_sb)
                    nc.vector.tensor_add(out=vrow[:, c0 + 1, 0:C], in0=pvB, in1=bv_sb)
                nc.vector.tensor_add(
                    out=vrow[0:1, 0, 0:C], in0=vrow[0:1, 0, 0:C], in1=ev_sb[0:1, :]
                )
                nc.vector.tensor_add(
                    out=vrow[96:128, NMC - 1, 0:C],
                    in0=vrow[96:128, NMC - 1, 0:C],
                    in1=ev_sb[96:128, :],
                )
                if dbg:
                    nc.sync.dma_start(out=dq_d[:, :], in_=qT[:, :])
                    nc.sync.dma_start(out=dk_d[:, :], in_=kT[:, :])
                    nc.sync.dma_start(out=dv_d[:, :], in_=vrow[:, :, :])

            # ---------------- attention ----------------
            with tc.tile_pool(name="stg", bufs=2, space="PSUM") as stg, tc.tile_pool(
                name="ups", bufs=1, space="PSUM"
            ) as ups, tc.tile_pool(name="esb", bufs=3) as esb, tc.tile_pool(
                name="osb", bufs=2
            ) as osb, tc.tile_pool(name="drp", bufs=2, space="DRAM") as drp:
                for g in range(NG):
                    ua = ups.tile([128, G], F32, tag="ua", name="ua")
                    ub = ups.tile([128, G], F32, tag="ub", name="ub")
                    qs_lo = qT[0:64, g * G : (g + 1) * G]
                    qs_hi = qT[64:128, g * G : (g + 1) * G]
                    prev = None
                    c = 0
                    for gs in GROUPS:
                        st = stg.tile([128, 3 * G], F32, tag="st", name="st")
                        for i in range(0, gs, 2):
                            ca = c + i
                            nc.tensor.matmul(
                                st[:, i * G : (i + 1) * G],
                                lhsT=kT[0:64, ca * MC : (ca + 1) * MC],
                                rhs=qs_lo,
                                start=True,
                                stop=True,
                                tile_position=(0, 0),
                            )
                            if i + 1 < gs:
                                cb = c + i + 1
                                nc.tensor.matmul(
                                    st[:, (i + 1) * G : (i + 2) * G],
                                    lhsT=kT[64:128, cb * MC : (cb + 1) * MC],
                                    rhs=qs_hi,
                                    start=True,
                                    stop=True,
                                    tile_position=(64, 0),
                                )
                        es = esb.tile([128, 3 * G], F32, tag="es", name="es")
                        nc.scalar.activation(
                            out=es[:, : gs * G], in_=st[:, : gs * G], func=EXP
                        )
                        if dbg and g == 0 and c == 0:
                            nc.sync.dma_start(out=des_d[:, : gs * G], in_=es[:, : gs * G])
                        if prev is not None:
                            pes, pc, pgs = prev
                            for i in range(pgs):
                                cc = pc + i
                                nc.tensor.matmul(
                                    ua[0:65, :],
                                    lhsT=vrow[0:64, cc, :],
                                    rhs=pes[0:64, i * G : (i + 1) * G],
                                    start=(cc == 0),
                                    stop=False,
                                    tile_position=(0, 0),
                                )
                                nc.tensor.matmul(
                                    ub[0:65, :],
                                    lhsT=vrow[64:128, cc, :],
                                    rhs=pes[64:128, i * G : (i + 1) * G],
                                    start=(cc == 0),
                                    stop=False,
                                    tile_position=(64, 0),
                                )
                        prev = (es, c, gs)
                        c += gs
                    pes, pc, pgs = prev
                    for i in range(pgs):
                        cc = pc + i
                        nc.tensor.matmul(
                            ua[0:65, :],
                            lhsT=vrow[0:64, cc, :],
                            rhs=pes[0:64, i * G : (i + 1) * G],
                            start=False,
                            stop=(cc == NMC - 1),
                            tile_position=(0, 0),
                        )
                        nc.tensor.matmul(
                            ub[0:65, :],
                            lhsT=vrow[64:128, cc, :],
                            rhs=pes[64:128, i * G : (i + 1) * G],
                            start=False,
                            stop=(cc == NMC - 1),
                            tile_position=(64, 0),
                        )

                    # normalize: usum = ua + ub ; out^T = usum[:64] / usum[64]
                    # (DVE may read only one PSUM operand per instruction)
                    ubs = osb.tile([65, G], F32, tag="ubs", name="ubs")
                    nc.vector.tensor_copy(out=ubs, in_=ub[0:65, :])
                    usum = osb.tile([65, G], F32, tag="us", name="usum")
                    nc.vector.tensor_add(out=usum, in0=ua[0:65, :], in1=ubs)
                    rec = osb.tile([65, G], F32, tag="rc", name="rec")
                    nc.vector.reciprocal(out=rec[64:65, :], in_=usum[64:65, :])
                    # partition-broadcast via DRAM bounce (custom GPSIMD bcast
                    # ucode does not honor the partition-64 source AP on HW)
                    rb = drp.tile([1, G], F32, tag="rb", name="rb")
                    nc.sync.dma_start(out=rb, in_=rec[64:65, :])
                    r64 = osb.tile([64, G], F32, tag="r64", name="r64")
                    nc.sync.dma_start(out=r64, in_=rb[:, :].to_broadcast((64, G)))
                    if dbg and g == 0:
                        nc.sync.dma_start(out=dus_d[:, :], in_=usum[:, :])
                        nc.sync.dma_start(out=dr_d[:, :], in_=r64[:, :])
                    outT = osb.tile([64, G], F32, tag="ot", name="outT")
                    nc.vector.tensor_mul(out=outT, in0=usum[0:64, :], in1=r64)

                    # output projection: y^T = wo^T @ out^T + bo
                    yp = ups.tile([128, G], F32, tag="ua", name="yp")
                    nc.tensor.matmul(
                        yp[0:64, :],
                        lhsT=wo_sb,
                        rhs=outT,
                        start=True,
                        stop=True,
                        tile_position=(0, 0),
                    )
                    ysb = osb.tile([64, G], F32, tag="y", name="ysb")
                    nc.vector.tensor_scalar_add(out=ysb, in0=yp[0:64, :], scalar1=bo_sb)
                    nc.sync.dma_start(out=yt_d[:, g * G : (g + 1) * G], in_=ysb)

    nc.compile()
    return nc


_NC_CACHE = None


def _get_program():
    global _NC_CACHE
    if _NC_CACHE is None:
        _NC_CACHE = build_program()
    return _NC_CACHE


def make_in_maps(Q, K, V, wq, bq, wk, bk, wv, bv, wd, bd, wo, bo):
    f32 = np.float32

    def fold(w1):
        return np.stack([w1[0].astype(f32) @ wd[k].astype(f32) for k in range(3)], 0)

    wqd, wkd, wvd = fold(wq), fold(wk), fold(wv)
    sum_wd = (wd[0] + wd[1] + wd[2]).astype(f32)

    def fold_bias(b1):
        return (b1.astype(f32) @ sum_wd + bd.astype(f32)).astype(f32)

    bqd, bkd, bvd = fold_bias(bq), fold_bias(bk), fold_bias(bv)

    def stack_w_qk(w3):
        # [128, 3, 128]: halves duplicated on partitions, col-duplicated weights
        out = np.zeros((128, 3, 128), f32)
        for h in range(2):
            for k in range(3):
                out[64 * h : 64 * h + 64, k, 0:64] = w3[k]
                out[64 * h : 64 * h + 64, k, 64:128] = w3[k]
        return out

    def stack_w_v(w3):
        out = np.zeros((128, 3, C), f32)
        for h in range(2):
            for k in range(3):
                out[64 * h : 64 * h + 64, k, :] = w3[k]
        return out

    def dup_col(b):
        return np.tile(b.astype(f32).reshape(C, 1), (2, 1))

    wq3 = stack_w_qk(wqd)
    wk3 = stack_w_qk(wkd)
    wv3 = stack_w_v(wvd)
    wo2 = np.ascontiguousarray(wo[0].astype(f32))
    bq2 = dup_col(bqd)
    bk2 = dup_col(bkd)
    bvb = np.tile(bvd.reshape(1, C), (128, 1)).astype(f32)
    bot = bo.astype(f32).reshape(C, 1)

    def edges(b1):
        e0 = -(b1.astype(f32) @ wd[0].astype(f32))
        e1 = -(b1.astype(f32) @ wd[2].astype(f32))
        return e0.astype(f32), e1.astype(f32)

    eq0, eq1 = edges(bq)
    ek0, ek1 = edges(bk)
    ev0, ev1 = edges(bv)
    eq2 = np.tile(np.stack([eq0, eq1], 1), (2, 1)).astype(f32)
    ek2 = np.tile(np.stack([ek0, ek1], 1), (2, 1)).astype(f32)
    ev2 = np.stack([ev0, ev1], 0).astype(f32)

    shared = dict(
        wq3=wq3, wk3=wk3, wv3=wv3, wo2=wo2, bq2=bq2, bk2=bk2, bvb=bvb, bot=bot,
        eq2=eq2, ek2=ek2, ev2=ev2,
    )

    def padT(x):
        z = np.zeros((C, L + 2), f32)
        z[:, 1 : L + 1] = x.astype(f32).T
        return z

    in_maps = []
    for b in range(B):
        m = dict(shared)
        m["qt"] = padT(Q[b])
        m["kt"] = padT(K[b])
        m["vt"] = padT(V[b])
        in_maps.append(m)
    return in_maps


def kernel(**inputs):
    Q = np.asarray(inputs["Q"], np.float32)
    K = np.asarray(inputs["K"], np.float32)
    V = np.asarray(inputs["V"], np.float32)
    in_maps = make_in_maps(
        Q, K, V,
        np.asarray(inputs["wq"], np.float32), np.asarray(inputs["bq"], np.float32),
        np.asarray(inputs["wk"], np.float32), np.asarray(inputs["bk"], np.float32),
        np.asarray(inputs["wv"], np.float32), np.asarray(inputs["bv"], np.float32),
        np.asarray(inputs["wd"], np.float32), np.asarray(inputs["bd"], np.float32),
        np.asarray(inputs["wo"], np.float32), np.asarray(inputs["bo"], np.float32),
    )
    nc = _get_program()
    res = run_bass_kernel_spmd(nc, in_maps, core_ids=list(range(NCORES)))
    out = np.empty((B, L, C), np.float32)
    for b in range(B):
        out[b] = res.results[b]["yt"].T
    return out
